# revision 29
# baseline (speedup 1.0000x reference)
"""Trainium2 Bass kernel for windowed (block-diagonal) multi-head video attention.

Problem: x:[2,8192,1024] -> qkv proj -> 3D-window (2,8,8) attention over a
(8,32,32) token grid, 16 heads x 64 dim -> out proj -> [2,8192,1024].

Sharding: 8 cores, data-parallel over (batch, t-window-group).  Token order is
(t,h,w)-major, so the slab x[b, it*2048:(it+1)*2048, :] is contiguous and holds
exactly the 16 independent (h,w)-windows with t in {2it, 2it+1}.

Kernel structure (all matmul operands bf16; psum f32):
  - DMA-gather each window's 128 tokens as [128,1024], PE-transpose to x^T
    (contraction dim on partitions), evict to bf16.
  - QKV projection with host-pretransposed bf16 weights (softmax SCALE folded
    into the Q columns).  Q,K evicted head-transposed [hd, tok]; V token-major
    with a per-head ones column (65-stride) appended.
  - Per (window, head): S^T = K_h Q_h^T (K=64), exp on ACT -> E bf16.
  - A.V flipped: stationary E_h, moving v65 -> token-major O[q, h*65]; the
    ones column makes column 64 of each head the softmax denominator, which
    lands per-PARTITION -> DVE reciprocal [128,4] + one stride-0-broadcast
    tensor_tensor per head-bank fuses the normalize into the psum eviction.
    This removes the Ln/exp/PE-broadcast/copy chain of the old scheme.
  - O re-transposed on PE (bf16, cheap) into owT [c, tok] for the projection.
  - Out projection at ap=512, evict on ACT, DMA-scatter to token order.
Group-level software pipeline: group g's QK/V matmuls are interleaved with
group g-1's attention in PE program order, so the in-order PE queue always has
ready work while ACT/DVE chase the softmax chain.
"""

import sys

for _p in ("/opt/trn_rl_repo",):
    if _p not in sys.path:
        sys.path.insert(0, _p)

import numpy as np
import ml_dtypes

B, T, H, W = 2, 8, 32, 32
C, NH, HD = 1024, 16, 64
WT, WH, WW = 2, 8, 8
N = T * H * W              # 8192 tokens
SCALE = HD ** -0.5
NCORES = 8
SLAB = N // (T // WT)      # 2048 tokens per (b, it) slab
NWIN = (H // WH) * (W // WW)   # 16 windows per slab
M = WT * WH * WW           # 128 tokens per window
KC = C // 128              # 8 contraction chunks

_BUILD_CACHE = {}


def _split_drain_waits(nc, mybir, cap=1, event_cap=2):
    """This walrus build accepts only one sem wait per TPB instruction
    (Tile's scheduler attaches up to 3).  Move the excess onto
    InstEventSemaphore carriers (which hold 2) inserted right before the
    over-subscribed instruction on the same engine — the engine blocks on the
    carriers first, so semantics are unchanged."""
    for f in nc.m.functions:
        for bb in f.blocks:
            i = 0
            while i < len(bb.instructions):
                ins = bb.instructions[i]
                si = ins.sync_info
                my_cap = (
                    event_cap
                    if type(ins).__name__ == "InstEventSemaphore"
                    else cap
                )
                if si is not None and si.on_wait and len(si.on_wait) > my_cap:
                    waits = list(si.on_wait)
                    si.on_wait = waits[:my_cap]
                    extra = waits[my_cap:]
                    carriers = []
                    while extra:
                        chunk, extra = extra[:event_cap], extra[event_cap:]
                        ev = mybir.InstEventSemaphore(
                            name=f"I-{nc.next_id()}-waitsplit", ins=[], outs=[]
                        )
                        ev.engine = ins.engine
                        ev.sync_info = mybir.SyncInfo(
                            on_wait=list(chunk), on_update=[]
                        )
                        nc.register_instruction(ev)
                        carriers.append(ev)
                    bb.instructions[i:i] = carriers
                    i += len(carriers)
                i += 1


def _build(has_qkvb, has_projb):
    import concourse.bass as bass
    import concourse.tile as tile
    from concourse import mybir
    f32 = mybir.dt.float32
    fpr = mybir.dt.float32r
    bf16 = mybir.dt.bfloat16

    nc = bass.Bass("TRN2", target_bir_lowering=False, debug=False)
    # x pre-gathered to windows AND pre-transposed on the host:
    # xswT[win, p, k*128+t] = x_slab[token(win, t), c = 128*k + p]
    xswT = nc.dram_tensor(
        "xswT", [NWIN, 128, KC * 128], bf16, kind="ExternalInput"
    )
    # QK weights bank-major ([bank, p, k, 256oc]) and V weights ([nk, p, k,
    # 512oc]) so each bank's DMA is one contiguous 4/8KB-per-partition copy.
    wqk = nc.dram_tensor("wqk", [8, 128, KC * 256], bf16, kind="ExternalInput")
    wv = nc.dram_tensor("wv", [2, 128, KC * 512], bf16, kind="ExternalInput")
    projT = nc.dram_tensor("projT", [C, C], bf16, kind="ExternalInput")
    if has_qkvb:
        qkvb = nc.dram_tensor("qkvb", [1, 3 * C], bf16, kind="ExternalInput")
    if has_projb:
        projb = nc.dram_tensor("projb", [1, C], bf16, kind="ExternalInput")
    ident_d = nc.dram_tensor("ident", [128, 128], bf16, kind="ExternalInput")
    out = nc.dram_tensor("out", [SLAB, C], f32, kind="ExternalOutput")

    # output scatter view: slab token idx = tt*1024 + hh*32 + ww in a
    # [2, (4,8), (4,8)] = (tt, ih hh, iw ww) decomposition; window = (ih, iw)
    out_v = out.ap().rearrange(
        "(tt ih hh iw ww) c -> ih iw tt hh ww c", tt=WT, ih=4, hh=WH, iw=4, ww=WW
    )

    GW = 2
    TOKG = 128 * GW
    NGRP = NWIN // GW

    with tile.TileContext(nc) as tc:
        with (
            tc.tile_pool(name="wq", bufs=1) as wq_pool,
            tc.tile_pool(name="wp", bufs=1) as wp_pool,
            tc.tile_pool(name="const", bufs=1) as const_pool,
            tc.tile_pool(name="xT", bufs=2) as xT_pool,
            tc.tile_pool(name="qk", bufs=2) as qk_pool,
            tc.tile_pool(name="v65", bufs=2) as v_pool,
            tc.tile_pool(name="E", bufs=12) as e_pool,
            tc.tile_pool(name="rq", bufs=8) as rq_pool,
            tc.tile_pool(name="Osb", bufs=2) as o_sb_pool,
            tc.tile_pool(name="owT", bufs=2) as ow_pool,
            tc.tile_pool(name="o", bufs=2) as o_pool,
            tc.tile_pool(name="psA", bufs=4, space="PSUM") as psA,
            tc.tile_pool(name="psS", bufs=2, space="PSUM") as psS_pool,
            tc.tile_pool(name="psO", bufs=2, space="PSUM") as psO_pool,
        ):
            # identity via DMA: make_identity runs on GpSimd, whose cold
            # start would gate the first PE transpose
            ident_bf = const_pool.tile([128, 128], bf16)
            nc.scalar.dma_start(ident_bf[:], ident_d.ap())

            # QK weights split across both DMA queues in psum-bank consumption
            # order (0,4,1,5,2,6,3,7), so bank b's matmuls start as soon as
            # its slice lands and neither queue serializes the full 4MB.
            # Each DMA is one contiguous 4KB-per-partition transfer.
            wq_sb = wq_pool.tile([128, 8, KC, 256], bf16)
            for bank in (0, 4, 1, 5):
                nc.sync.dma_start(
                    wq_sb[:, bank],
                    wqk.ap()[bank].rearrange("p (k o) -> p k o", o=256),
                )
            wv_sb = wp_pool.tile([128, 2, KC, 512], bf16, name="wv_sb")
            wp_sb = wp_pool.tile([128, KC, C], bf16)
            wp_src = projT.ap().rearrange("(k p) o -> p k o", p=128)

            def emit_late_weights(stage):
                # Late weights ride the scalar queue behind group 0/1's x^T
                # gathers, ordered by first use: QK banks 2,6,3,7, V columns
                # (V(0) runs ~25us in), then the proj weight (step 1).
                if stage == 0:
                    for bank in (2, 6, 3, 7):
                        nc.scalar.dma_start(
                            wq_sb[:, bank],
                            wqk.ap()[bank].rearrange("p (k o) -> p k o", o=256),
                        )
                    for nk in range(2):
                        nc.scalar.dma_start(
                            wv_sb[:, nk],
                            wv.ap()[nk].rearrange("p (k o) -> p k o", o=512),
                        )
                else:
                    for k in range(KC):
                        nc.scalar.dma_start(
                            wp_sb[:, k : k + 1, :], wp_src[:, k : k + 1, :]
                        )
            if has_qkvb or has_projb:
                onesf = const_pool.tile([1, TOKG], f32)
                nc.vector.memset(onesf[:], 1.0)
                ones = const_pool.tile([1, TOKG], bf16)
                nc.scalar.copy(ones[:], onesf[:])
            if has_qkvb:
                qkvb_sb = const_pool.tile([1, 3 * C], bf16)
                nc.sync.dma_start(qkvb_sb[:], qkvb.ap())
            if has_projb:
                projb_sb = const_pool.tile([1, C], bf16)
                nc.sync.dma_start(projb_sb[:], projb.ap())

            # ---- per-group emission helpers (pipelined main loop below) ----

            def emit_gather(grp):
                """DMA the host-pretransposed x^T for the group's 2 windows:
                contiguous 2KB-per-partition transfers, no on-chip work."""
                wins = [divmod(GW * grp + w, 4) for w in range(GW)]
                xT = xT_pool.tile([128, GW, KC, 128], bf16)
                for w in range(GW):
                    nc.scalar.dma_start(
                        xT[:, w], xswT.ap()[GW * grp + w].rearrange(
                            "p (k t) -> p k t", t=128
                        )
                    )
                return xT, wins

            def emit_qk(grp, xT, qkT, banks):
                """QK projection for the given psum banks (bank = 2 oc-chunks,
                oc-chunks 0..7 = Q heads, 8..15 = K heads).  Evict to the
                64-partition per-head layout qkT[64, head-slot, tok] (slot
                h for Q, NH+h for K); SCALE is folded into the host weights."""
                qkTv = qkT[:].rearrange("p (s two) t -> p s two t", two=2)
                for bank in banks:
                    ps = psA.tile([128, 512], f32, tag="psA")
                    for sub in range(2):
                        oc = 2 * bank + sub
                        for k in range(KC):
                            nc.tensor.matmul(
                                ps[:, TOKG * sub : TOKG * (sub + 1)],
                                wq_sb[:, bank, k, 128 * sub : 128 * sub + 128],
                                xT[:, :, k, :],
                                start=(k == 0),
                                stop=(k == KC - 1 and not has_qkvb),
                            )
                        if has_qkvb:
                            nc.tensor.matmul(
                                ps[:, TOKG * sub : TOKG * (sub + 1)],
                                qkvb_sb[0:1, 128 * oc : 128 * (oc + 1)],
                                ones[0:1, 0:TOKG],
                                start=False,
                                stop=True,
                            )
                    psv = ps[:].rearrange("p (c t) -> p c t", t=TOKG)
                    with nc.allow_low_precision(reason="bf16 eviction"):
                        nc.vector.tensor_copy(
                            qkTv[:, 2 * bank : 2 * bank + 2, 0, :], psv[0:64]
                        )
                        nc.vector.tensor_copy(
                            qkTv[:, 2 * bank : 2 * bank + 2, 1, :], psv[64:128]
                        )

            def emit_v(grp, xT):
                """V projection, token-major per window, ap=512; ones column
                per head (65-stride) for the fused softmax denominator."""
                v65 = v_pool.tile([128, GW, NH, HD + 1], bf16)
                nc.vector.memset(v65[:, :, :, HD : HD + 1], 1.0)
                for w in range(GW):
                    for nk in range(2):
                        ps = psA.tile([128, 512], f32, tag="psA")
                        for k in range(KC):
                            nc.tensor.matmul(
                                ps[:],
                                xT[:, w, k, :],
                                wv_sb[:, nk, k, :],
                                start=(k == 0),
                                stop=(k == KC - 1 and not has_qkvb),
                            )
                        if has_qkvb:
                            nc.tensor.matmul(
                                ps[:],
                                ones[0:1, 0:128],
                                qkvb_sb[0:1, 2 * C + 512 * nk : 2 * C + 512 * (nk + 1)],
                                start=False,
                                stop=True,
                            )
                        # one strided eviction for all 8 heads of this bank —
                        # on DVE: the ACT queue must stay exp-only, else psA
                        # recycling stalls the PE behind serialized ACT ops
                        with nc.allow_low_precision(reason="bf16 eviction"):
                            nc.vector.tensor_copy(
                                v65[:, w, 8 * nk : 8 * nk + 8, 0:HD],
                                ps[:].rearrange("p (h e) -> p h e", e=HD),
                            )
                return v65

            def emit_s(qkT, w, E_tiles):
                """Score matmuls S^T = K_h Q_h^T for one window (4 head-banks
                of 4 heads), exp on ACT into bf16 E tiles."""
                for hb in range(4):
                    psS = psS_pool.tile([128, 512], f32, tag="psS")
                    for m in range(4):
                        h = 4 * hb + m
                        nc.tensor.matmul(
                            psS[:, 128 * m : 128 * (m + 1)],
                            qkT[:, NH + h, 128 * w : 128 * (w + 1)],
                            qkT[:, h, 128 * w : 128 * (w + 1)],
                            start=True,
                            stop=True,
                        )
                    E = e_pool.tile([128, 512], bf16, tag="E")
                    with nc.allow_low_precision(reason="bf16 attn weights"):
                        nc.scalar.activation(
                            E[:], psS[:], mybir.ActivationFunctionType.Exp
                        )
                    E_tiles.append(E)

            def emit_av(v65, w, E_tiles, O_sb):
                """A.V with E stationary: token-major O[q, h, d]; column 64 of
                each head is the softmax denominator (per-partition!), so a
                DVE reciprocal + stride-0-broadcast multiply fuses the
                normalization into the psum eviction."""
                for hb in range(4):
                    E = E_tiles[hb]
                    psO = psO_pool.tile([128, 4, HD + 1], f32, tag="psO")
                    for m in range(4):
                        h = 4 * hb + m
                        nc.tensor.matmul(
                            psO[:, m, :],
                            E[:, 128 * m : 128 * (m + 1)],
                            v65[:, w, h, :],
                            start=True,
                            stop=True,
                        )
                    rq = rq_pool.tile([128, 4], f32, tag="rq")
                    nc.vector.reciprocal(rq[:], psO[:, :, HD])
                    with nc.allow_low_precision(reason="bf16 attn out"):
                        nc.vector.tensor_tensor(
                            O_sb[:, 4 * hb : 4 * hb + 4, :],
                            psO[:, :, 0:HD],
                            rq[:].unsqueeze(-1).broadcast_to([128, 4, HD]),
                            op=mybir.AluOpType.mult,
                        )

            def emit_ot(O_sb, owT):
                """Re-transpose token-major O into owT[c, tok] for the proj."""
                O_flat = O_sb[:].rearrange("p h d -> p (h d)")
                for tb in range(2):
                    ps = psA.tile([128, 512], bf16, tag="psA")
                    for j in range(4):
                        jj = 4 * tb + j
                        nc.tensor.transpose(
                            ps[:, 128 * j : 128 * (j + 1)],
                            O_flat[:, 128 * jj : 128 * (jj + 1)],
                            ident_bf[:],
                        )
                    nc.vector.tensor_copy(
                        owT[:, 4 * tb : 4 * tb + 4, :],
                        ps[:].rearrange("p (c t) -> p c t", t=128),
                    )

            def emit_proj(owT, ih, iw):
                """Out projection (ap=512) and DMA-scatter to token order."""
                otile = o_pool.tile([128, C], f32)
                for nk in range(2):
                    ps = psA.tile([128, 512], f32, tag="psA")
                    lo = 512 * nk
                    for k in range(KC):
                        nc.tensor.matmul(
                            ps[:],
                            owT[:, k, :],
                            wp_sb[:, k, lo : lo + 512],
                            start=(k == 0),
                            stop=(k == KC - 1 and not has_projb),
                        )
                    if has_projb:
                        nc.tensor.matmul(
                            ps[:],
                            ones[0:1, 0:128],
                            projb_sb[0:1, lo : lo + 512],
                            start=False,
                            stop=True,
                        )
                    nc.vector.tensor_copy(otile[:, lo : lo + 512], ps[:])
                for tt in range(WT):
                    nc.sync.dma_start(
                        out_v[ih, iw, tt], otile[64 * tt : 64 * (tt + 1), :]
                    )

            # ---- pipelined main loop: attention trails QKV by one group ----
            prev = None  # (qkT, v65, wins) of group g-1
            for g in range(NGRP + 1):
                cur_xT = cur_wins = None
                if g < NGRP:
                    cur_xT, cur_wins = emit_gather(g)
                    if g <= 1:
                        emit_late_weights(g)
                    qkT = qk_pool.tile([64, 4 * KC, TOKG], bf16)

                pE = [[], []]
                pO = [None, None]
                pOw = [None, None]
                if prev is not None:
                    pqkT, pv65, pwins = prev
                    for w in range(GW):
                        emit_s(pqkT, w, pE[w])
                if g < NGRP:
                    emit_qk(g, cur_xT, qkT, banks=(0, 4, 1, 5))
                if prev is not None:
                    for w in range(GW):
                        pO[w] = o_sb_pool.tile([128, NH, HD], bf16, tag="Osb", name="Osb")
                        emit_av(pv65, w, pE[w], pO[w])
                if g < NGRP:
                    emit_qk(g, cur_xT, qkT, banks=(2, 6))
                if prev is not None:
                    for w in range(GW):
                        pOw[w] = ow_pool.tile([128, KC, 128], bf16, tag="owT", name="owT")
                        emit_ot(pO[w], pOw[w])
                if g < NGRP:
                    emit_qk(g, cur_xT, qkT, banks=(3, 7))
                if prev is not None:
                    for w, (ih, iw) in enumerate(pwins):
                        emit_proj(pOw[w], ih, iw)
                if g < NGRP:
                    v65 = emit_v(g, cur_xT)
                    prev = (qkT, v65, cur_wins)

    _split_drain_waits(nc, mybir)
    return nc


def _get_nc(has_qkvb, has_projb):
    key = (has_qkvb, has_projb)
    if key not in _BUILD_CACHE:
        _BUILD_CACHE[key] = _build(has_qkvb, has_projb)
    return _BUILD_CACHE[key]


def _host_weights(qkv_w, proj_w):
    """Pre-transpose weights, fold the softmax scale into the Q columns,
    cast to bf16, and pre-bank QK ([bank, p, k*256]) / V ([nk, p, k*512])
    so each on-chip weight DMA is a contiguous per-partition copy."""
    wqkvT = qkv_w.T.astype(np.float32)
    wqkvT[:, :C] *= SCALE
    wqkvT = wqkvT.astype(ml_dtypes.bfloat16)
    # [ (k p), oc ] -> [p, k, oc]
    wt = wqkvT.reshape(KC, 128, 3 * C).transpose(1, 0, 2)
    wqk = np.ascontiguousarray(
        wt[:, :, : 2 * C].reshape(128, KC, 8, 256).transpose(2, 0, 1, 3)
    ).reshape(8, 128, KC * 256)
    wv = np.ascontiguousarray(
        wt[:, :, 2 * C :].reshape(128, KC, 2, 512).transpose(2, 0, 1, 3)
    ).reshape(2, 128, KC * 512)
    return wqk, wv, np.ascontiguousarray(proj_w.T).astype(ml_dtypes.bfloat16)


def _host_gather_transpose(x):
    """x [B, N, C] -> per (b, it) slab: window-gathered, channel-transposed
    bf16 [NWIN, 128, KC*128]: out[win, 128*?+p? ...] — precisely
    out[win, p, 128*k + t] = x[b, slab + token(win, t), 128*k + p], with
    token (tt, hh, ww) order inside the window and win = 4*ih + iw."""
    xb = x.astype(ml_dtypes.bfloat16)
    xb = xb.reshape(B, T // WT, WT, 4, WH, 4, WW, KC, 128)
    # -> [b, it, ih, iw, (tt hh ww)=t, k, p] -> [b, it, win, p, k, t]
    xb = xb.transpose(0, 1, 3, 5, 2, 4, 6, 7, 8)
    xb = xb.reshape(B, T // WT, NWIN, M, KC, 128)
    xb = np.ascontiguousarray(xb.transpose(0, 1, 2, 5, 4, 3))
    return xb.reshape(B, T // WT, NWIN, 128, KC * 128)


def make_in_maps(x, qkv_w, qkv_b, proj_w, proj_b):
    has_qkvb = bool(np.any(qkv_b))
    has_projb = bool(np.any(proj_b))
    wqk, wv, projT = _host_weights(qkv_w, proj_w)
    xswT = _host_gather_transpose(x)
    ident = np.eye(128, dtype=np.float32).astype(ml_dtypes.bfloat16)
    in_maps = []
    for core in range(NCORES):
        b, it = divmod(core, T // WT)
        im = {
            "xswT": xswT[b, it],
            "wqk": wqk,
            "wv": wv,
            "projT": projT,
            "ident": ident,
        }
        if has_qkvb:
            im["qkvb"] = qkv_b.reshape(1, 3 * C).astype(ml_dtypes.bfloat16)
        if has_projb:
            im["projb"] = proj_b.reshape(1, C).astype(ml_dtypes.bfloat16)
        in_maps.append(im)
    return in_maps, has_qkvb, has_projb


def kernel(x, qkv_w, qkv_b, proj_w, proj_b, t, h, w, **_unused):
    from concourse.bass_utils import run_bass_kernel_spmd

    x = np.asarray(x, dtype=np.float32)
    qkv_w = np.asarray(qkv_w, dtype=np.float32)
    qkv_b = np.asarray(qkv_b, dtype=np.float32)
    proj_w = np.asarray(proj_w, dtype=np.float32)
    proj_b = np.asarray(proj_b, dtype=np.float32)
    assert x.shape == (B, N, C), x.shape
    assert int(t) == T and int(h) == H and int(w) == W

    in_maps, has_qkvb, has_projb = make_in_maps(x, qkv_w, qkv_b, proj_w, proj_b)
    nc = _get_nc(has_qkvb, has_projb)

    res = run_bass_kernel_spmd(nc, in_maps, core_ids=list(range(NCORES)))

    y = np.empty((B, N, C), dtype=np.float32)
    for core in range(NCORES):
        b, it = divmod(core, T // WT)
        y[b, it * SLAB : (it + 1) * SLAB, :] = res.results[core]["out"]
    return y


# revision 40
# speedup vs baseline: 1.1246x; 1.1246x over previous
"""Trainium2 Bass kernel for windowed (block-diagonal) multi-head video attention.

Problem: x:[2,8192,1024] -> qkv proj -> 3D-window (2,8,8) attention over a
(8,32,32) token grid, 16 heads x 64 dim -> out proj -> [2,8192,1024].

Sharding: 8 cores, data-parallel over (batch, t-window-group).  Token order is
(t,h,w)-major, so the slab x[b, it*2048:(it+1)*2048, :] is contiguous and holds
exactly the 16 independent (h,w)-windows with t in {2it, 2it+1}.

Kernel structure (all matmul operands bf16; psum f32):
  - DMA-gather each window's 128 tokens as [128,1024], PE-transpose to x^T
    (contraction dim on partitions), evict to bf16.
  - QKV projection with host-pretransposed bf16 weights (softmax SCALE folded
    into the Q columns).  Q,K evicted head-transposed [hd, tok]; V token-major
    with a per-head ones column (65-stride) appended.
  - Per (window, head): S^T = K_h Q_h^T (K=64), exp on ACT -> E bf16.
  - A.V flipped: stationary E_h, moving v65 -> token-major O[q, h*65]; the
    ones column makes column 64 of each head the softmax denominator, which
    lands per-PARTITION -> DVE reciprocal [128,4] + one stride-0-broadcast
    tensor_tensor per head-bank fuses the normalize into the psum eviction.
    This removes the Ln/exp/PE-broadcast/copy chain of the old scheme.
  - O re-transposed on PE (bf16, cheap) into owT [c, tok] for the projection.
  - Out projection at ap=512, evict on ACT, DMA-scatter to token order.
Group-level software pipeline: group g's QK/V matmuls are interleaved with
group g-1's attention in PE program order, so the in-order PE queue always has
ready work while ACT/DVE chase the softmax chain.
"""

import sys

for _p in ("/opt/trn_rl_repo",):
    if _p not in sys.path:
        sys.path.insert(0, _p)

import numpy as np
import ml_dtypes

B, T, H, W = 2, 8, 32, 32
C, NH, HD = 1024, 16, 64
WT, WH, WW = 2, 8, 8
N = T * H * W              # 8192 tokens
SCALE = HD ** -0.5
NCORES = 8
SLAB = N // (T // WT)      # 2048 tokens per (b, it) slab
NWIN = (H // WH) * (W // WW)   # 16 windows per slab
M = WT * WH * WW           # 128 tokens per window
KC = C // 128              # 8 contraction chunks

_BUILD_CACHE = {}


def _split_drain_waits(nc, mybir, cap=1, event_cap=2):
    """This walrus build accepts only one sem wait per TPB instruction
    (Tile's scheduler attaches up to 3).  Move the excess onto
    InstEventSemaphore carriers (which hold 2) inserted right before the
    over-subscribed instruction on the same engine — the engine blocks on the
    carriers first, so semantics are unchanged."""
    for f in nc.m.functions:
        for bb in f.blocks:
            i = 0
            while i < len(bb.instructions):
                ins = bb.instructions[i]
                si = ins.sync_info
                my_cap = (
                    event_cap
                    if type(ins).__name__ == "InstEventSemaphore"
                    else cap
                )
                if si is not None and si.on_wait and len(si.on_wait) > my_cap:
                    waits = list(si.on_wait)
                    si.on_wait = waits[:my_cap]
                    extra = waits[my_cap:]
                    carriers = []
                    while extra:
                        chunk, extra = extra[:event_cap], extra[event_cap:]
                        ev = mybir.InstEventSemaphore(
                            name=f"I-{nc.next_id()}-waitsplit", ins=[], outs=[]
                        )
                        ev.engine = ins.engine
                        ev.sync_info = mybir.SyncInfo(
                            on_wait=list(chunk), on_update=[]
                        )
                        nc.register_instruction(ev)
                        carriers.append(ev)
                    bb.instructions[i:i] = carriers
                    i += len(carriers)
                i += 1


def _build(has_qkvb, has_projb):
    import concourse.bass as bass
    import concourse.tile as tile
    from concourse import mybir
    f32 = mybir.dt.float32
    fpr = mybir.dt.float32r
    bf16 = mybir.dt.bfloat16

    nc = bass.Bass("TRN2", target_bir_lowering=False, debug=False)
    # x pre-gathered to windows AND pre-transposed on the host:
    # xswT[win, p, k*128+t] = x_slab[token(win, t), c = 128*k + p]
    xswT = nc.dram_tensor(
        "xswT", [NWIN, 128, KC * 128], bf16, kind="ExternalInput"
    )
    wqkvT = nc.dram_tensor("wqkvT", [C, 3 * C], bf16, kind="ExternalInput")
    projT = nc.dram_tensor("projT", [C, C], bf16, kind="ExternalInput")
    if has_qkvb:
        qkvb = nc.dram_tensor("qkvb", [1, 3 * C], bf16, kind="ExternalInput")
    if has_projb:
        projb = nc.dram_tensor("projb", [1, C], bf16, kind="ExternalInput")
    ident_d = nc.dram_tensor("ident", [128, 128], bf16, kind="ExternalInput")
    out = nc.dram_tensor("out", [SLAB, C], f32, kind="ExternalOutput")

    # output scatter view: slab token idx = tt*1024 + hh*32 + ww in a
    # [2, (4,8), (4,8)] = (tt, ih hh, iw ww) decomposition; window = (ih, iw)
    out_v = out.ap().rearrange(
        "(tt ih hh iw ww) c -> ih iw tt hh ww c", tt=WT, ih=4, hh=WH, iw=4, ww=WW
    )

    GW = 2
    TOKG = 128 * GW
    NGRP = NWIN // GW

    with tile.TileContext(nc) as tc:
        with (
            tc.tile_pool(name="wq", bufs=1) as wq_pool,
            tc.tile_pool(name="wp", bufs=1) as wp_pool,
            tc.tile_pool(name="const", bufs=1) as const_pool,
            tc.tile_pool(name="xT", bufs=2) as xT_pool,
            tc.tile_pool(name="qk", bufs=2) as qk_pool,
            tc.tile_pool(name="v65", bufs=2) as v_pool,
            tc.tile_pool(name="E", bufs=6) as e_pool,
            tc.tile_pool(name="rq", bufs=8) as rq_pool,
            tc.tile_pool(name="Osb", bufs=2) as o_sb_pool,
            tc.tile_pool(name="owT", bufs=2) as ow_pool,
            tc.tile_pool(name="o", bufs=2) as o_pool,
            tc.tile_pool(name="psA", bufs=4, space="PSUM") as psA,
            tc.tile_pool(name="psS", bufs=2, space="PSUM") as psS_pool,
            tc.tile_pool(name="psO", bufs=2, space="PSUM") as psO_pool,
        ):
            # identity via DMA: make_identity runs on GpSimd, whose cold
            # start would gate the first PE transpose
            ident_bf = const_pool.tile([128, 128], bf16)
            nc.scalar.dma_start(ident_bf[:], ident_d.ap())

            # QK columns of the qkv weight on the sync queue (needed first);
            # V columns + proj weight on the scalar queue.
            wq_sb = wq_pool.tile([128, KC, 3 * C], bf16)
            wq_src = wqkvT.ap().rearrange("(k p) o -> p k o", p=128)
            for k in range(KC):
                nc.sync.dma_start(
                    wq_sb[:, k : k + 1, 0 : 2 * C], wq_src[:, k : k + 1, 0 : 2 * C]
                )
            for k in range(KC):
                nc.scalar.dma_start(
                    wq_sb[:, k : k + 1, 2 * C :], wq_src[:, k : k + 1, 2 * C :]
                )
            wp_sb = wp_pool.tile([128, KC, C], bf16)
            wp_src = projT.ap().rearrange("(k p) o -> p k o", p=128)
            for k in range(KC):
                nc.scalar.dma_start(wp_sb[:, k : k + 1, :], wp_src[:, k : k + 1, :])
            if has_qkvb or has_projb:
                onesf = const_pool.tile([1, TOKG], f32)
                nc.vector.memset(onesf[:], 1.0)
                ones = const_pool.tile([1, TOKG], bf16)
                nc.scalar.copy(ones[:], onesf[:])
            if has_qkvb:
                qkvb_sb = const_pool.tile([1, 3 * C], bf16)
                nc.sync.dma_start(qkvb_sb[:], qkvb.ap())
            if has_projb:
                projb_sb = const_pool.tile([1, C], bf16)
                nc.sync.dma_start(projb_sb[:], projb.ap())

            # ---- per-group emission helpers (pipelined main loop below) ----

            def emit_gather(grp):
                """DMA the host-pretransposed x^T for the group's 2 windows:
                contiguous 2KB-per-partition transfers, no on-chip work."""
                wins = [divmod(GW * grp + w, 4) for w in range(GW)]
                xT = xT_pool.tile([128, GW, KC, 128], bf16)
                for w in range(GW):
                    nc.scalar.dma_start(
                        xT[:, w], xswT.ap()[GW * grp + w].rearrange(
                            "p (k t) -> p k t", t=128
                        )
                    )
                return xT, wins

            def emit_qk(grp, xT, qkT, banks):
                """QK projection for the given psum banks (bank = 2 oc-chunks,
                oc-chunks 0..7 = Q heads, 8..15 = K heads).  Evict to the
                64-partition per-head layout qkT[64, head-slot, tok] (slot
                h for Q, NH+h for K); SCALE is folded into the host weights."""
                qkTv = qkT[:].rearrange("p (s two) t -> p s two t", two=2)
                for bank in banks:
                    ps = psA.tile([128, 512], f32, tag="psA")
                    for sub in range(2):
                        oc = 2 * bank + sub
                        for k in range(KC):
                            nc.tensor.matmul(
                                ps[:, TOKG * sub : TOKG * (sub + 1)],
                                wq_sb[:, k, 128 * oc : 128 * (oc + 1)],
                                xT[:, :, k, :],
                                start=(k == 0),
                                stop=(k == KC - 1 and not has_qkvb),
                            )
                        if has_qkvb:
                            nc.tensor.matmul(
                                ps[:, TOKG * sub : TOKG * (sub + 1)],
                                qkvb_sb[0:1, 128 * oc : 128 * (oc + 1)],
                                ones[0:1, 0:TOKG],
                                start=False,
                                stop=True,
                            )
                    psv = ps[:].rearrange("p (c t) -> p c t", t=TOKG)
                    with nc.allow_low_precision(reason="bf16 eviction"):
                        nc.vector.tensor_copy(
                            qkTv[:, 2 * bank : 2 * bank + 2, 0, :], psv[0:64]
                        )
                        nc.vector.tensor_copy(
                            qkTv[:, 2 * bank : 2 * bank + 2, 1, :], psv[64:128]
                        )

            def emit_v(grp, xT):
                """V projection, token-major per window, ap=512; ones column
                per head (65-stride) for the fused softmax denominator."""
                v65 = v_pool.tile([128, GW, NH, HD + 1], bf16)
                nc.vector.memset(v65[:, :, :, HD : HD + 1], 1.0)
                for w in range(GW):
                    for nk in range(2):
                        ps = psA.tile([128, 512], f32, tag="psA")
                        lo = 2 * C + 512 * nk
                        for k in range(KC):
                            nc.tensor.matmul(
                                ps[:],
                                xT[:, w, k, :],
                                wq_sb[:, k, lo : lo + 512],
                                start=(k == 0),
                                stop=(k == KC - 1 and not has_qkvb),
                            )
                        if has_qkvb:
                            nc.tensor.matmul(
                                ps[:],
                                ones[0:1, 0:128],
                                qkvb_sb[0:1, 2 * C + 512 * nk : 2 * C + 512 * (nk + 1)],
                                start=False,
                                stop=True,
                            )
                        # one strided eviction for all 8 heads of this bank
                        nc.scalar.copy(
                            v65[:, w, 8 * nk : 8 * nk + 8, 0:HD],
                            ps[:].rearrange("p (h e) -> p h e", e=HD),
                        )
                return v65

            def emit_s(qkT, w, E_tiles):
                """Score matmuls S^T = K_h Q_h^T for one window (4 head-banks
                of 4 heads), exp on ACT into bf16 E tiles."""
                for hb in range(4):
                    psS = psS_pool.tile([128, 512], f32, tag="psS")
                    for m in range(4):
                        h = 4 * hb + m
                        nc.tensor.matmul(
                            psS[:, 128 * m : 128 * (m + 1)],
                            qkT[:, NH + h, 128 * w : 128 * (w + 1)],
                            qkT[:, h, 128 * w : 128 * (w + 1)],
                            start=True,
                            stop=True,
                        )
                    E = e_pool.tile([128, 512], bf16, tag="E")
                    with nc.allow_low_precision(reason="bf16 attn weights"):
                        nc.scalar.activation(
                            E[:], psS[:], mybir.ActivationFunctionType.Exp
                        )
                    E_tiles.append(E)

            def emit_av(v65, w, E_tiles, O_sb):
                """A.V with E stationary: token-major O[q, h, d]; column 64 of
                each head is the softmax denominator (per-partition!), so a
                DVE reciprocal + stride-0-broadcast multiply fuses the
                normalization into the psum eviction."""
                for hb in range(4):
                    E = E_tiles[hb]
                    psO = psO_pool.tile([128, 4, HD + 1], f32, tag="psO")
                    for m in range(4):
                        h = 4 * hb + m
                        nc.tensor.matmul(
                            psO[:, m, :],
                            E[:, 128 * m : 128 * (m + 1)],
                            v65[:, w, h, :],
                            start=True,
                            stop=True,
                        )
                    rq = rq_pool.tile([128, 4], f32, tag="rq")
                    nc.vector.reciprocal(rq[:], psO[:, :, HD])
                    with nc.allow_low_precision(reason="bf16 attn out"):
                        nc.vector.tensor_tensor(
                            O_sb[:, 4 * hb : 4 * hb + 4, :],
                            psO[:, :, 0:HD],
                            rq[:].unsqueeze(-1).broadcast_to([128, 4, HD]),
                            op=mybir.AluOpType.mult,
                        )

            def emit_ot(O_sb, owT):
                """Re-transpose token-major O into owT[c, tok] for the proj."""
                O_flat = O_sb[:].rearrange("p h d -> p (h d)")
                for tb in range(2):
                    ps = psA.tile([128, 512], bf16, tag="psA")
                    for j in range(4):
                        jj = 4 * tb + j
                        nc.tensor.transpose(
                            ps[:, 128 * j : 128 * (j + 1)],
                            O_flat[:, 128 * jj : 128 * (jj + 1)],
                            ident_bf[:],
                        )
                    nc.vector.tensor_copy(
                        owT[:, 4 * tb : 4 * tb + 4, :],
                        ps[:].rearrange("p (c t) -> p c t", t=128),
                    )

            def emit_proj(owT, ih, iw):
                """Out projection (ap=512), evict on the otherwise-idle Pool
                engine (keeps the ACT queue exp-only and DVE out of the psA
                recycling path), DMA-scatter to token order."""
                otile = o_pool.tile([128, C], f32)
                for nk in range(2):
                    ps = psA.tile([128, 512], f32, tag="psA")
                    lo = 512 * nk
                    for k in range(KC):
                        nc.tensor.matmul(
                            ps[:],
                            owT[:, k, :],
                            wp_sb[:, k, lo : lo + 512],
                            start=(k == 0),
                            stop=(k == KC - 1 and not has_projb),
                        )
                    if has_projb:
                        nc.tensor.matmul(
                            ps[:],
                            ones[0:1, 0:128],
                            projb_sb[0:1, lo : lo + 512],
                            start=False,
                            stop=True,
                        )
                    nc.scalar.copy(otile[:, lo : lo + 512], ps[:])
                for tt in range(WT):
                    nc.sync.dma_start(
                        out_v[ih, iw, tt], otile[64 * tt : 64 * (tt + 1), :]
                    )

            # ---- pipelined main loop: attention trails QKV by one group ----
            prev = None  # (qkT, v65, wins) of group g-1
            for g in range(NGRP + 1):
                cur_xT = cur_wins = None
                if g < NGRP:
                    cur_xT, cur_wins = emit_gather(g)
                    qkT = qk_pool.tile([64, 4 * KC, TOKG], bf16)

                pE = [[], []]
                pO = [None, None]
                pOw = [None, None]
                if prev is not None:
                    pqkT, pv65, pwins = prev
                    for w in range(GW):
                        emit_s(pqkT, w, pE[w])
                if g < NGRP:
                    emit_qk(g, cur_xT, qkT, banks=(0, 4, 1, 5))
                if prev is not None:
                    for w in range(GW):
                        pO[w] = o_sb_pool.tile([128, NH, HD], bf16, tag="Osb", name="Osb")
                        emit_av(pv65, w, pE[w], pO[w])
                if g < NGRP:
                    emit_qk(g, cur_xT, qkT, banks=(2, 6))
                if prev is not None:
                    for w in range(GW):
                        pOw[w] = ow_pool.tile([128, KC, 128], bf16, tag="owT", name="owT")
                        emit_ot(pO[w], pOw[w])
                if g < NGRP:
                    emit_qk(g, cur_xT, qkT, banks=(3, 7))
                if prev is not None:
                    for w, (ih, iw) in enumerate(pwins):
                        emit_proj(pOw[w], ih, iw)
                if g < NGRP:
                    v65 = emit_v(g, cur_xT)
                    prev = (qkT, v65, cur_wins)

    _split_drain_waits(nc, mybir)
    return nc


def _get_nc(has_qkvb, has_projb):
    key = (has_qkvb, has_projb)
    if key not in _BUILD_CACHE:
        _BUILD_CACHE[key] = _build(has_qkvb, has_projb)
    return _BUILD_CACHE[key]


def _host_weights(qkv_w, proj_w):
    """Pre-transpose weights, fold the softmax scale into the Q columns,
    cast to bf16 for the PE."""
    wqkvT = qkv_w.T.astype(np.float32)
    wqkvT[:, :C] *= SCALE
    return (
        np.ascontiguousarray(wqkvT).astype(ml_dtypes.bfloat16),
        np.ascontiguousarray(proj_w.T).astype(ml_dtypes.bfloat16),
    )


def _host_gather_transpose(x):
    """x [B, N, C] -> per (b, it) slab: window-gathered, channel-transposed
    bf16 [NWIN, 128, KC*128]: out[win, 128*?+p? ...] — precisely
    out[win, p, 128*k + t] = x[b, slab + token(win, t), 128*k + p], with
    token (tt, hh, ww) order inside the window and win = 4*ih + iw."""
    xb = x.astype(ml_dtypes.bfloat16)
    xb = xb.reshape(B, T // WT, WT, 4, WH, 4, WW, KC, 128)
    # -> [b, it, ih, iw, (tt hh ww)=t, k, p] -> [b, it, win, p, k, t]
    xb = xb.transpose(0, 1, 3, 5, 2, 4, 6, 7, 8)
    xb = xb.reshape(B, T // WT, NWIN, M, KC, 128)
    xb = np.ascontiguousarray(xb.transpose(0, 1, 2, 5, 4, 3))
    return xb.reshape(B, T // WT, NWIN, 128, KC * 128)


def make_in_maps(x, qkv_w, qkv_b, proj_w, proj_b):
    has_qkvb = bool(np.any(qkv_b))
    has_projb = bool(np.any(proj_b))
    wqkvT, projT = _host_weights(qkv_w, proj_w)
    xswT = _host_gather_transpose(x)
    ident = np.eye(128, dtype=np.float32).astype(ml_dtypes.bfloat16)
    in_maps = []
    for core in range(NCORES):
        b, it = divmod(core, T // WT)
        im = {
            "xswT": xswT[b, it],
            "wqkvT": wqkvT,
            "projT": projT,
            "ident": ident,
        }
        if has_qkvb:
            im["qkvb"] = qkv_b.reshape(1, 3 * C).astype(ml_dtypes.bfloat16)
        if has_projb:
            im["projb"] = proj_b.reshape(1, C).astype(ml_dtypes.bfloat16)
        in_maps.append(im)
    return in_maps, has_qkvb, has_projb


def kernel(x, qkv_w, qkv_b, proj_w, proj_b, t, h, w, **_unused):
    from concourse.bass_utils import run_bass_kernel_spmd

    x = np.asarray(x, dtype=np.float32)
    qkv_w = np.asarray(qkv_w, dtype=np.float32)
    qkv_b = np.asarray(qkv_b, dtype=np.float32)
    proj_w = np.asarray(proj_w, dtype=np.float32)
    proj_b = np.asarray(proj_b, dtype=np.float32)
    assert x.shape == (B, N, C), x.shape
    assert int(t) == T and int(h) == H and int(w) == W

    in_maps, has_qkvb, has_projb = make_in_maps(x, qkv_w, qkv_b, proj_w, proj_b)
    nc = _get_nc(has_qkvb, has_projb)

    res = run_bass_kernel_spmd(nc, in_maps, core_ids=list(range(NCORES)))

    y = np.empty((B, N, C), dtype=np.float32)
    for core in range(NCORES):
        b, it = divmod(core, T // WT)
        y[b, it * SLAB : (it + 1) * SLAB, :] = res.results[core]["out"]
    return y


# revision 41
# speedup vs baseline: 1.1961x; 1.0636x over previous
"""Trainium2 Bass kernel for windowed (block-diagonal) multi-head video attention.

Problem: x:[2,8192,1024] -> qkv proj -> 3D-window (2,8,8) attention over a
(8,32,32) token grid, 16 heads x 64 dim -> out proj -> [2,8192,1024].

Sharding: 8 cores, data-parallel over (batch, t-window-group).  Token order is
(t,h,w)-major, so the slab x[b, it*2048:(it+1)*2048, :] is contiguous and holds
exactly the 16 independent (h,w)-windows with t in {2it, 2it+1}.

Kernel structure (all matmul operands bf16; psum f32):
  - DMA-gather each window's 128 tokens as [128,1024], PE-transpose to x^T
    (contraction dim on partitions), evict to bf16.
  - QKV projection with host-pretransposed bf16 weights (softmax SCALE folded
    into the Q columns).  Q,K evicted head-transposed [hd, tok]; V token-major
    with a per-head ones column (65-stride) appended.
  - Per (window, head): S^T = K_h Q_h^T (K=64), exp on ACT -> E bf16.
  - A.V flipped: stationary E_h, moving v65 -> token-major O[q, h*65]; the
    ones column makes column 64 of each head the softmax denominator, which
    lands per-PARTITION -> DVE reciprocal [128,4] + one stride-0-broadcast
    tensor_tensor per head-bank fuses the normalize into the psum eviction.
    This removes the Ln/exp/PE-broadcast/copy chain of the old scheme.
  - O re-transposed on PE (bf16, cheap) into owT [c, tok] for the projection.
  - Out projection at ap=512, evict on ACT, DMA-scatter to token order.
Group-level software pipeline: group g's QK/V matmuls are interleaved with
group g-1's attention in PE program order, so the in-order PE queue always has
ready work while ACT/DVE chase the softmax chain.
"""

import sys

for _p in ("/opt/trn_rl_repo",):
    if _p not in sys.path:
        sys.path.insert(0, _p)

import numpy as np
import ml_dtypes

B, T, H, W = 2, 8, 32, 32
C, NH, HD = 1024, 16, 64
WT, WH, WW = 2, 8, 8
N = T * H * W              # 8192 tokens
SCALE = HD ** -0.5
NCORES = 8
SLAB = N // (T // WT)      # 2048 tokens per (b, it) slab
NWIN = (H // WH) * (W // WW)   # 16 windows per slab
M = WT * WH * WW           # 128 tokens per window
KC = C // 128              # 8 contraction chunks

_BUILD_CACHE = {}


def _split_drain_waits(nc, mybir, cap=1, event_cap=2):
    """This walrus build accepts only one sem wait per TPB instruction
    (Tile's scheduler attaches up to 3).  Move the excess onto
    InstEventSemaphore carriers (which hold 2) inserted right before the
    over-subscribed instruction on the same engine — the engine blocks on the
    carriers first, so semantics are unchanged."""
    for f in nc.m.functions:
        for bb in f.blocks:
            i = 0
            while i < len(bb.instructions):
                ins = bb.instructions[i]
                si = ins.sync_info
                my_cap = (
                    event_cap
                    if type(ins).__name__ == "InstEventSemaphore"
                    else cap
                )
                if si is not None and si.on_wait and len(si.on_wait) > my_cap:
                    waits = list(si.on_wait)
                    si.on_wait = waits[:my_cap]
                    extra = waits[my_cap:]
                    carriers = []
                    while extra:
                        chunk, extra = extra[:event_cap], extra[event_cap:]
                        ev = mybir.InstEventSemaphore(
                            name=f"I-{nc.next_id()}-waitsplit", ins=[], outs=[]
                        )
                        ev.engine = ins.engine
                        ev.sync_info = mybir.SyncInfo(
                            on_wait=list(chunk), on_update=[]
                        )
                        nc.register_instruction(ev)
                        carriers.append(ev)
                    bb.instructions[i:i] = carriers
                    i += len(carriers)
                i += 1


def _build(has_qkvb, has_projb):
    import concourse.bass as bass
    import concourse.tile as tile
    from concourse import mybir
    f32 = mybir.dt.float32
    fpr = mybir.dt.float32r
    bf16 = mybir.dt.bfloat16

    nc = bass.Bass("TRN2", target_bir_lowering=False, debug=False)
    # x pre-gathered to windows AND pre-transposed on the host:
    # xswT[win, p, k*128+t] = x_slab[token(win, t), c = 128*k + p]
    xswT = nc.dram_tensor(
        "xswT", [NWIN, 128, KC * 128], bf16, kind="ExternalInput"
    )
    wqkvT = nc.dram_tensor("wqkvT", [C, 3 * C], bf16, kind="ExternalInput")
    projT = nc.dram_tensor("projT", [C, C], bf16, kind="ExternalInput")
    if has_qkvb:
        qkvb = nc.dram_tensor("qkvb", [1, 3 * C], bf16, kind="ExternalInput")
    if has_projb:
        projb = nc.dram_tensor("projb", [1, C], bf16, kind="ExternalInput")
    ident_d = nc.dram_tensor("ident", [128, 128], bf16, kind="ExternalInput")
    out = nc.dram_tensor("out", [SLAB, C], f32, kind="ExternalOutput")

    # output scatter view: slab token idx = tt*1024 + hh*32 + ww in a
    # [2, (4,8), (4,8)] = (tt, ih hh, iw ww) decomposition; window = (ih, iw)
    out_v = out.ap().rearrange(
        "(tt ih hh iw ww) c -> ih iw tt hh ww c", tt=WT, ih=4, hh=WH, iw=4, ww=WW
    )

    GW = 2
    TOKG = 128 * GW
    NGRP = NWIN // GW

    with tile.TileContext(nc) as tc:
        with (
            tc.tile_pool(name="wq", bufs=1) as wq_pool,
            tc.tile_pool(name="wp", bufs=1) as wp_pool,
            tc.tile_pool(name="const", bufs=1) as const_pool,
            tc.tile_pool(name="xT", bufs=2) as xT_pool,
            tc.tile_pool(name="qk", bufs=2) as qk_pool,
            tc.tile_pool(name="v65", bufs=2) as v_pool,
            tc.tile_pool(name="E", bufs=6) as e_pool,
            tc.tile_pool(name="rq", bufs=8) as rq_pool,
            tc.tile_pool(name="Osb", bufs=2) as o_sb_pool,
            tc.tile_pool(name="owT", bufs=2) as ow_pool,
            tc.tile_pool(name="o", bufs=2) as o_pool,
            tc.tile_pool(name="psA", bufs=4, space="PSUM") as psA,
            tc.tile_pool(name="psS", bufs=2, space="PSUM") as psS_pool,
            tc.tile_pool(name="psO", bufs=2, space="PSUM") as psO_pool,
        ):
            # identity via DMA: make_identity runs on GpSimd, whose cold
            # start would gate the first PE transpose
            ident_bf = const_pool.tile([128, 128], bf16)
            nc.scalar.dma_start(ident_bf[:], ident_d.ap())

            # All weights on the sync queue, in consumption order: QK weight
            # columns bank-major (matching the QK psum bank order so bank b's
            # matmuls start as soon as its slice lands), then V columns, then
            # the proj weight.  The scalar queue carries only the per-group
            # x^T gathers, so group 0's tokens don't queue behind weights.
            wq_sb = wq_pool.tile([128, KC, 3 * C], bf16)
            wq_src = wqkvT.ap().rearrange("(k p) o -> p k o", p=128)
            for bank in (0, 4, 1, 5, 2, 6, 3, 7):
                lo = 256 * bank
                nc.sync.dma_start(
                    wq_sb[:, :, lo : lo + 256], wq_src[:, :, lo : lo + 256]
                )
            for bank in range(2):
                lo = 2 * C + 512 * bank
                nc.sync.dma_start(
                    wq_sb[:, :, lo : lo + 512], wq_src[:, :, lo : lo + 512]
                )
            wp_sb = wp_pool.tile([128, KC, C], bf16)
            wp_src = projT.ap().rearrange("(k p) o -> p k o", p=128)
            for k in range(KC):
                nc.sync.dma_start(wp_sb[:, k : k + 1, :], wp_src[:, k : k + 1, :])
            if has_qkvb or has_projb:
                onesf = const_pool.tile([1, TOKG], f32)
                nc.vector.memset(onesf[:], 1.0)
                ones = const_pool.tile([1, TOKG], bf16)
                nc.scalar.copy(ones[:], onesf[:])
            if has_qkvb:
                qkvb_sb = const_pool.tile([1, 3 * C], bf16)
                nc.sync.dma_start(qkvb_sb[:], qkvb.ap())
            if has_projb:
                projb_sb = const_pool.tile([1, C], bf16)
                nc.sync.dma_start(projb_sb[:], projb.ap())

            # ---- per-group emission helpers (pipelined main loop below) ----

            def emit_gather(grp):
                """DMA the host-pretransposed x^T for the group's 2 windows:
                contiguous 2KB-per-partition transfers, no on-chip work."""
                wins = [divmod(GW * grp + w, 4) for w in range(GW)]
                xT = xT_pool.tile([128, GW, KC, 128], bf16)
                for w in range(GW):
                    nc.scalar.dma_start(
                        xT[:, w], xswT.ap()[GW * grp + w].rearrange(
                            "p (k t) -> p k t", t=128
                        )
                    )
                return xT, wins

            def emit_qk(grp, xT, qkT, banks):
                """QK projection for the given psum banks (bank = 2 oc-chunks,
                oc-chunks 0..7 = Q heads, 8..15 = K heads).  Evict to the
                64-partition per-head layout qkT[64, head-slot, tok] (slot
                h for Q, NH+h for K); SCALE is folded into the host weights."""
                qkTv = qkT[:].rearrange("p (s two) t -> p s two t", two=2)
                for bank in banks:
                    ps = psA.tile([128, 512], f32, tag="psA")
                    for sub in range(2):
                        oc = 2 * bank + sub
                        for k in range(KC):
                            nc.tensor.matmul(
                                ps[:, TOKG * sub : TOKG * (sub + 1)],
                                wq_sb[:, k, 128 * oc : 128 * (oc + 1)],
                                xT[:, :, k, :],
                                start=(k == 0),
                                stop=(k == KC - 1 and not has_qkvb),
                            )
                        if has_qkvb:
                            nc.tensor.matmul(
                                ps[:, TOKG * sub : TOKG * (sub + 1)],
                                qkvb_sb[0:1, 128 * oc : 128 * (oc + 1)],
                                ones[0:1, 0:TOKG],
                                start=False,
                                stop=True,
                            )
                    psv = ps[:].rearrange("p (c t) -> p c t", t=TOKG)
                    with nc.allow_low_precision(reason="bf16 eviction"):
                        nc.vector.tensor_copy(
                            qkTv[:, 2 * bank : 2 * bank + 2, 0, :], psv[0:64]
                        )
                        nc.vector.tensor_copy(
                            qkTv[:, 2 * bank : 2 * bank + 2, 1, :], psv[64:128]
                        )

            def emit_v(grp, xT):
                """V projection, token-major per window, ap=512; ones column
                per head (65-stride) for the fused softmax denominator."""
                v65 = v_pool.tile([128, GW, NH, HD + 1], bf16)
                nc.vector.memset(v65[:, :, :, HD : HD + 1], 1.0)
                for w in range(GW):
                    for nk in range(2):
                        ps = psA.tile([128, 512], f32, tag="psA")
                        lo = 2 * C + 512 * nk
                        for k in range(KC):
                            nc.tensor.matmul(
                                ps[:],
                                xT[:, w, k, :],
                                wq_sb[:, k, lo : lo + 512],
                                start=(k == 0),
                                stop=(k == KC - 1 and not has_qkvb),
                            )
                        if has_qkvb:
                            nc.tensor.matmul(
                                ps[:],
                                ones[0:1, 0:128],
                                qkvb_sb[0:1, 2 * C + 512 * nk : 2 * C + 512 * (nk + 1)],
                                start=False,
                                stop=True,
                            )
                        # one strided eviction for all 8 heads of this bank
                        nc.scalar.copy(
                            v65[:, w, 8 * nk : 8 * nk + 8, 0:HD],
                            ps[:].rearrange("p (h e) -> p h e", e=HD),
                        )
                return v65

            def emit_s(qkT, w, E_tiles):
                """Score matmuls S^T = K_h Q_h^T for one window (4 head-banks
                of 4 heads), exp on ACT into bf16 E tiles."""
                for hb in range(4):
                    psS = psS_pool.tile([128, 512], f32, tag="psS")
                    for m in range(4):
                        h = 4 * hb + m
                        nc.tensor.matmul(
                            psS[:, 128 * m : 128 * (m + 1)],
                            qkT[:, NH + h, 128 * w : 128 * (w + 1)],
                            qkT[:, h, 128 * w : 128 * (w + 1)],
                            start=True,
                            stop=True,
                        )
                    E = e_pool.tile([128, 512], bf16, tag="E")
                    with nc.allow_low_precision(reason="bf16 attn weights"):
                        nc.scalar.activation(
                            E[:], psS[:], mybir.ActivationFunctionType.Exp
                        )
                    E_tiles.append(E)

            def emit_av(v65, w, E_tiles, O_sb):
                """A.V with E stationary: token-major O[q, h, d]; column 64 of
                each head is the softmax denominator (per-partition!), so a
                DVE reciprocal + stride-0-broadcast multiply fuses the
                normalization into the psum eviction."""
                for hb in range(4):
                    E = E_tiles[hb]
                    psO = psO_pool.tile([128, 4, HD + 1], f32, tag="psO")
                    for m in range(4):
                        h = 4 * hb + m
                        nc.tensor.matmul(
                            psO[:, m, :],
                            E[:, 128 * m : 128 * (m + 1)],
                            v65[:, w, h, :],
                            start=True,
                            stop=True,
                        )
                    rq = rq_pool.tile([128, 4], f32, tag="rq")
                    nc.vector.reciprocal(rq[:], psO[:, :, HD])
                    with nc.allow_low_precision(reason="bf16 attn out"):
                        nc.vector.tensor_tensor(
                            O_sb[:, 4 * hb : 4 * hb + 4, :],
                            psO[:, :, 0:HD],
                            rq[:].unsqueeze(-1).broadcast_to([128, 4, HD]),
                            op=mybir.AluOpType.mult,
                        )

            def emit_ot(O_sb, owT):
                """Re-transpose token-major O into owT[c, tok] for the proj."""
                O_flat = O_sb[:].rearrange("p h d -> p (h d)")
                for tb in range(2):
                    ps = psA.tile([128, 512], bf16, tag="psA")
                    for j in range(4):
                        jj = 4 * tb + j
                        nc.tensor.transpose(
                            ps[:, 128 * j : 128 * (j + 1)],
                            O_flat[:, 128 * jj : 128 * (jj + 1)],
                            ident_bf[:],
                        )
                    nc.vector.tensor_copy(
                        owT[:, 4 * tb : 4 * tb + 4, :],
                        ps[:].rearrange("p (c t) -> p c t", t=128),
                    )

            def emit_proj(owT, ih, iw):
                """Out projection (ap=512), evict on the otherwise-idle Pool
                engine (keeps the ACT queue exp-only and DVE out of the psA
                recycling path), DMA-scatter to token order."""
                otile = o_pool.tile([128, C], f32)
                for nk in range(2):
                    ps = psA.tile([128, 512], f32, tag="psA")
                    lo = 512 * nk
                    for k in range(KC):
                        nc.tensor.matmul(
                            ps[:],
                            owT[:, k, :],
                            wp_sb[:, k, lo : lo + 512],
                            start=(k == 0),
                            stop=(k == KC - 1 and not has_projb),
                        )
                    if has_projb:
                        nc.tensor.matmul(
                            ps[:],
                            ones[0:1, 0:128],
                            projb_sb[0:1, lo : lo + 512],
                            start=False,
                            stop=True,
                        )
                    nc.scalar.copy(otile[:, lo : lo + 512], ps[:])
                for tt in range(WT):
                    nc.sync.dma_start(
                        out_v[ih, iw, tt], otile[64 * tt : 64 * (tt + 1), :]
                    )

            # ---- pipelined main loop: attention trails QKV by one group ----
            prev = None  # (qkT, v65, wins) of group g-1
            for g in range(NGRP + 1):
                cur_xT = cur_wins = None
                if g < NGRP:
                    cur_xT, cur_wins = emit_gather(g)
                    qkT = qk_pool.tile([64, 4 * KC, TOKG], bf16)

                pE = [[], []]
                pO = [None, None]
                pOw = [None, None]
                if prev is not None:
                    pqkT, pv65, pwins = prev
                    for w in range(GW):
                        emit_s(pqkT, w, pE[w])
                if g < NGRP:
                    emit_qk(g, cur_xT, qkT, banks=(0, 4, 1, 5))
                if prev is not None:
                    for w in range(GW):
                        pO[w] = o_sb_pool.tile([128, NH, HD], bf16, tag="Osb", name="Osb")
                        emit_av(pv65, w, pE[w], pO[w])
                if g < NGRP:
                    emit_qk(g, cur_xT, qkT, banks=(2, 6))
                if prev is not None:
                    for w in range(GW):
                        pOw[w] = ow_pool.tile([128, KC, 128], bf16, tag="owT", name="owT")
                        emit_ot(pO[w], pOw[w])
                if g < NGRP:
                    emit_qk(g, cur_xT, qkT, banks=(3, 7))
                if prev is not None:
                    for w, (ih, iw) in enumerate(pwins):
                        emit_proj(pOw[w], ih, iw)
                if g < NGRP:
                    v65 = emit_v(g, cur_xT)
                    prev = (qkT, v65, cur_wins)

    _split_drain_waits(nc, mybir)
    return nc


def _get_nc(has_qkvb, has_projb):
    key = (has_qkvb, has_projb)
    if key not in _BUILD_CACHE:
        _BUILD_CACHE[key] = _build(has_qkvb, has_projb)
    return _BUILD_CACHE[key]


def _host_weights(qkv_w, proj_w):
    """Pre-transpose weights, fold the softmax scale into the Q columns,
    cast to bf16 for the PE."""
    wqkvT = qkv_w.T.astype(np.float32)
    wqkvT[:, :C] *= SCALE
    return (
        np.ascontiguousarray(wqkvT).astype(ml_dtypes.bfloat16),
        np.ascontiguousarray(proj_w.T).astype(ml_dtypes.bfloat16),
    )


def _host_gather_transpose(x):
    """x [B, N, C] -> per (b, it) slab: window-gathered, channel-transposed
    bf16 [NWIN, 128, KC*128]: out[win, 128*?+p? ...] — precisely
    out[win, p, 128*k + t] = x[b, slab + token(win, t), 128*k + p], with
    token (tt, hh, ww) order inside the window and win = 4*ih + iw."""
    xb = x.astype(ml_dtypes.bfloat16)
    xb = xb.reshape(B, T // WT, WT, 4, WH, 4, WW, KC, 128)
    # -> [b, it, ih, iw, (tt hh ww)=t, k, p] -> [b, it, win, p, k, t]
    xb = xb.transpose(0, 1, 3, 5, 2, 4, 6, 7, 8)
    xb = xb.reshape(B, T // WT, NWIN, M, KC, 128)
    xb = np.ascontiguousarray(xb.transpose(0, 1, 2, 5, 4, 3))
    return xb.reshape(B, T // WT, NWIN, 128, KC * 128)


def make_in_maps(x, qkv_w, qkv_b, proj_w, proj_b):
    has_qkvb = bool(np.any(qkv_b))
    has_projb = bool(np.any(proj_b))
    wqkvT, projT = _host_weights(qkv_w, proj_w)
    xswT = _host_gather_transpose(x)
    ident = np.eye(128, dtype=np.float32).astype(ml_dtypes.bfloat16)
    in_maps = []
    for core in range(NCORES):
        b, it = divmod(core, T // WT)
        im = {
            "xswT": xswT[b, it],
            "wqkvT": wqkvT,
            "projT": projT,
            "ident": ident,
        }
        if has_qkvb:
            im["qkvb"] = qkv_b.reshape(1, 3 * C).astype(ml_dtypes.bfloat16)
        if has_projb:
            im["projb"] = proj_b.reshape(1, C).astype(ml_dtypes.bfloat16)
        in_maps.append(im)
    return in_maps, has_qkvb, has_projb


def kernel(x, qkv_w, qkv_b, proj_w, proj_b, t, h, w, **_unused):
    from concourse.bass_utils import run_bass_kernel_spmd

    x = np.asarray(x, dtype=np.float32)
    qkv_w = np.asarray(qkv_w, dtype=np.float32)
    qkv_b = np.asarray(qkv_b, dtype=np.float32)
    proj_w = np.asarray(proj_w, dtype=np.float32)
    proj_b = np.asarray(proj_b, dtype=np.float32)
    assert x.shape == (B, N, C), x.shape
    assert int(t) == T and int(h) == H and int(w) == W

    in_maps, has_qkvb, has_projb = make_in_maps(x, qkv_w, qkv_b, proj_w, proj_b)
    nc = _get_nc(has_qkvb, has_projb)

    res = run_bass_kernel_spmd(nc, in_maps, core_ids=list(range(NCORES)))

    y = np.empty((B, N, C), dtype=np.float32)
    for core in range(NCORES):
        b, it = divmod(core, T // WT)
        y[b, it * SLAB : (it + 1) * SLAB, :] = res.results[core]["out"]
    return y


# revision 42
# speedup vs baseline: 1.2007x; 1.0039x over previous
"""Trainium2 Bass kernel for windowed (block-diagonal) multi-head video attention.

Problem: x:[2,8192,1024] -> qkv proj -> 3D-window (2,8,8) attention over a
(8,32,32) token grid, 16 heads x 64 dim -> out proj -> [2,8192,1024].

Sharding: 8 cores, data-parallel over (batch, t-window-group).  Token order is
(t,h,w)-major, so the slab x[b, it*2048:(it+1)*2048, :] is contiguous and holds
exactly the 16 independent (h,w)-windows with t in {2it, 2it+1}.

Kernel structure (all matmul operands bf16; psum f32):
  - DMA-gather each window's 128 tokens as [128,1024], PE-transpose to x^T
    (contraction dim on partitions), evict to bf16.
  - QKV projection with host-pretransposed bf16 weights (softmax SCALE folded
    into the Q columns).  Q,K evicted head-transposed [hd, tok]; V token-major
    with a per-head ones column (65-stride) appended.
  - Per (window, head): S^T = K_h Q_h^T (K=64), exp on ACT -> E bf16.
  - A.V flipped: stationary E_h, moving v65 -> token-major O[q, h*65]; the
    ones column makes column 64 of each head the softmax denominator, which
    lands per-PARTITION -> DVE reciprocal [128,4] + one stride-0-broadcast
    tensor_tensor per head-bank fuses the normalize into the psum eviction.
    This removes the Ln/exp/PE-broadcast/copy chain of the old scheme.
  - O re-transposed on PE (bf16, cheap) into owT [c, tok] for the projection.
  - Out projection at ap=512, evict on ACT, DMA-scatter to token order.
Group-level software pipeline: group g's QK/V matmuls are interleaved with
group g-1's attention in PE program order, so the in-order PE queue always has
ready work while ACT/DVE chase the softmax chain.
"""

import sys

for _p in ("/opt/trn_rl_repo",):
    if _p not in sys.path:
        sys.path.insert(0, _p)

import numpy as np
import ml_dtypes

B, T, H, W = 2, 8, 32, 32
C, NH, HD = 1024, 16, 64
WT, WH, WW = 2, 8, 8
N = T * H * W              # 8192 tokens
SCALE = HD ** -0.5
NCORES = 8
SLAB = N // (T // WT)      # 2048 tokens per (b, it) slab
NWIN = (H // WH) * (W // WW)   # 16 windows per slab
M = WT * WH * WW           # 128 tokens per window
KC = C // 128              # 8 contraction chunks

_BUILD_CACHE = {}


def _split_drain_waits(nc, mybir, cap=1, event_cap=2):
    """This walrus build accepts only one sem wait per TPB instruction
    (Tile's scheduler attaches up to 3).  Move the excess onto
    InstEventSemaphore carriers (which hold 2) inserted right before the
    over-subscribed instruction on the same engine — the engine blocks on the
    carriers first, so semantics are unchanged."""
    for f in nc.m.functions:
        for bb in f.blocks:
            i = 0
            while i < len(bb.instructions):
                ins = bb.instructions[i]
                si = ins.sync_info
                my_cap = (
                    event_cap
                    if type(ins).__name__ == "InstEventSemaphore"
                    else cap
                )
                if si is not None and si.on_wait and len(si.on_wait) > my_cap:
                    waits = list(si.on_wait)
                    si.on_wait = waits[:my_cap]
                    extra = waits[my_cap:]
                    carriers = []
                    while extra:
                        chunk, extra = extra[:event_cap], extra[event_cap:]
                        ev = mybir.InstEventSemaphore(
                            name=f"I-{nc.next_id()}-waitsplit", ins=[], outs=[]
                        )
                        ev.engine = ins.engine
                        ev.sync_info = mybir.SyncInfo(
                            on_wait=list(chunk), on_update=[]
                        )
                        nc.register_instruction(ev)
                        carriers.append(ev)
                    bb.instructions[i:i] = carriers
                    i += len(carriers)
                i += 1


def _build(has_qkvb, has_projb):
    import concourse.bass as bass
    import concourse.tile as tile
    from concourse import mybir
    f32 = mybir.dt.float32
    fpr = mybir.dt.float32r
    bf16 = mybir.dt.bfloat16

    nc = bass.Bass("TRN2", target_bir_lowering=False, debug=False)
    # x pre-gathered to windows AND pre-transposed on the host:
    # xswT[win, p, k*128+t] = x_slab[token(win, t), c = 128*k + p]
    xswT = nc.dram_tensor(
        "xswT", [NWIN, 128, KC * 128], bf16, kind="ExternalInput"
    )
    # QK weights bank-major ([bank, p, k, 256oc]) and V weights ([nk, p, k,
    # 512oc]) so each weight DMA is one contiguous per-partition copy.
    wqk = nc.dram_tensor("wqk", [8, 128, KC * 256], bf16, kind="ExternalInput")
    wv = nc.dram_tensor("wv", [2, 128, KC * 512], bf16, kind="ExternalInput")
    projT = nc.dram_tensor("projT", [C, C], bf16, kind="ExternalInput")
    if has_qkvb:
        qkvb = nc.dram_tensor("qkvb", [1, 3 * C], bf16, kind="ExternalInput")
    if has_projb:
        projb = nc.dram_tensor("projb", [1, C], bf16, kind="ExternalInput")
    ident_d = nc.dram_tensor("ident", [128, 128], bf16, kind="ExternalInput")
    out = nc.dram_tensor("out", [SLAB, C], f32, kind="ExternalOutput")

    # output scatter view: slab token idx = tt*1024 + hh*32 + ww in a
    # [2, (4,8), (4,8)] = (tt, ih hh, iw ww) decomposition; window = (ih, iw)
    out_v = out.ap().rearrange(
        "(tt ih hh iw ww) c -> ih iw tt hh ww c", tt=WT, ih=4, hh=WH, iw=4, ww=WW
    )

    GW = 2
    TOKG = 128 * GW
    NGRP = NWIN // GW

    with tile.TileContext(nc) as tc:
        with (
            tc.tile_pool(name="wq", bufs=1) as wq_pool,
            tc.tile_pool(name="wp", bufs=1) as wp_pool,
            tc.tile_pool(name="const", bufs=1) as const_pool,
            tc.tile_pool(name="xT", bufs=2) as xT_pool,
            tc.tile_pool(name="qk", bufs=2) as qk_pool,
            tc.tile_pool(name="v65", bufs=2) as v_pool,
            tc.tile_pool(name="E", bufs=6) as e_pool,
            tc.tile_pool(name="rq", bufs=8) as rq_pool,
            tc.tile_pool(name="Osb", bufs=2) as o_sb_pool,
            tc.tile_pool(name="owT", bufs=2) as ow_pool,
            tc.tile_pool(name="o", bufs=2) as o_pool,
            tc.tile_pool(name="psA", bufs=4, space="PSUM") as psA,
            tc.tile_pool(name="psS", bufs=2, space="PSUM") as psS_pool,
            tc.tile_pool(name="psO", bufs=2, space="PSUM") as psO_pool,
        ):
            # identity via DMA: make_identity runs on GpSimd, whose cold
            # start would gate the first PE transpose
            ident_bf = const_pool.tile([128, 128], bf16)
            nc.scalar.dma_start(ident_bf[:], ident_d.ap())

            # All weights on the sync queue, in consumption order: QK weight
            # columns bank-major (matching the QK psum bank order so bank b's
            # matmuls start as soon as its slice lands), then V columns, then
            # the proj weight.  The scalar queue carries only the per-group
            # x^T gathers, so group 0's tokens don't queue behind weights.
            wq_sb = wq_pool.tile([128, 8, KC, 256], bf16)
            for bank in (0, 4, 1, 5, 2, 6, 3, 7):
                nc.sync.dma_start(
                    wq_sb[:, bank],
                    wqk.ap()[bank].rearrange("p (k o) -> p k o", o=256),
                )
            wv_sb = wq_pool.tile([128, 2, KC, 512], bf16, name="wv_sb")
            for nk in range(2):
                nc.sync.dma_start(
                    wv_sb[:, nk],
                    wv.ap()[nk].rearrange("p (k o) -> p k o", o=512),
                )
            wp_sb = wp_pool.tile([128, KC, C], bf16)
            wp_src = projT.ap().rearrange("(k p) o -> p k o", p=128)
            for k in range(KC):
                nc.sync.dma_start(wp_sb[:, k : k + 1, :], wp_src[:, k : k + 1, :])
            if has_qkvb or has_projb:
                onesf = const_pool.tile([1, TOKG], f32)
                nc.vector.memset(onesf[:], 1.0)
                ones = const_pool.tile([1, TOKG], bf16)
                nc.scalar.copy(ones[:], onesf[:])
            if has_qkvb:
                qkvb_sb = const_pool.tile([1, 3 * C], bf16)
                nc.sync.dma_start(qkvb_sb[:], qkvb.ap())
            if has_projb:
                projb_sb = const_pool.tile([1, C], bf16)
                nc.sync.dma_start(projb_sb[:], projb.ap())

            # ---- per-group emission helpers (pipelined main loop below) ----

            def emit_gather(grp):
                """DMA the host-pretransposed x^T for the group's 2 windows:
                contiguous 2KB-per-partition transfers, no on-chip work."""
                wins = [divmod(GW * grp + w, 4) for w in range(GW)]
                xT = xT_pool.tile([128, GW, KC, 128], bf16)
                for w in range(GW):
                    nc.scalar.dma_start(
                        xT[:, w], xswT.ap()[GW * grp + w].rearrange(
                            "p (k t) -> p k t", t=128
                        )
                    )
                return xT, wins

            def emit_qk(grp, xT, qkT, banks):
                """QK projection for the given psum banks (bank = 2 oc-chunks,
                oc-chunks 0..7 = Q heads, 8..15 = K heads).  Evict to the
                64-partition per-head layout qkT[64, head-slot, tok] (slot
                h for Q, NH+h for K); SCALE is folded into the host weights."""
                qkTv = qkT[:].rearrange("p (s two) t -> p s two t", two=2)
                for bank in banks:
                    ps = psA.tile([128, 512], f32, tag="psA")
                    for sub in range(2):
                        oc = 2 * bank + sub
                        for k in range(KC):
                            nc.tensor.matmul(
                                ps[:, TOKG * sub : TOKG * (sub + 1)],
                                wq_sb[:, bank, k, 128 * sub : 128 * sub + 128],
                                xT[:, :, k, :],
                                start=(k == 0),
                                stop=(k == KC - 1 and not has_qkvb),
                            )
                        if has_qkvb:
                            nc.tensor.matmul(
                                ps[:, TOKG * sub : TOKG * (sub + 1)],
                                qkvb_sb[0:1, 128 * oc : 128 * (oc + 1)],
                                ones[0:1, 0:TOKG],
                                start=False,
                                stop=True,
                            )
                    psv = ps[:].rearrange("p (c t) -> p c t", t=TOKG)
                    with nc.allow_low_precision(reason="bf16 eviction"):
                        nc.vector.tensor_copy(
                            qkTv[:, 2 * bank : 2 * bank + 2, 0, :], psv[0:64]
                        )
                        nc.vector.tensor_copy(
                            qkTv[:, 2 * bank : 2 * bank + 2, 1, :], psv[64:128]
                        )

            def emit_v(grp, xT):
                """V projection, token-major per window, ap=512; ones column
                per head (65-stride) for the fused softmax denominator."""
                v65 = v_pool.tile([128, GW, NH, HD + 1], bf16)
                nc.vector.memset(v65[:, :, :, HD : HD + 1], 1.0)
                for w in range(GW):
                    for nk in range(2):
                        ps = psA.tile([128, 512], f32, tag="psA")
                        for k in range(KC):
                            nc.tensor.matmul(
                                ps[:],
                                xT[:, w, k, :],
                                wv_sb[:, nk, k, :],
                                start=(k == 0),
                                stop=(k == KC - 1 and not has_qkvb),
                            )
                        if has_qkvb:
                            nc.tensor.matmul(
                                ps[:],
                                ones[0:1, 0:128],
                                qkvb_sb[0:1, 2 * C + 512 * nk : 2 * C + 512 * (nk + 1)],
                                start=False,
                                stop=True,
                            )
                        # one strided eviction for all 8 heads of this bank
                        nc.scalar.copy(
                            v65[:, w, 8 * nk : 8 * nk + 8, 0:HD],
                            ps[:].rearrange("p (h e) -> p h e", e=HD),
                        )
                return v65

            def emit_s(qkT, w, E_tiles):
                """Score matmuls S^T = K_h Q_h^T for one window (4 head-banks
                of 4 heads), exp on ACT into bf16 E tiles."""
                for hb in range(4):
                    psS = psS_pool.tile([128, 512], f32, tag="psS")
                    for m in range(4):
                        h = 4 * hb + m
                        nc.tensor.matmul(
                            psS[:, 128 * m : 128 * (m + 1)],
                            qkT[:, NH + h, 128 * w : 128 * (w + 1)],
                            qkT[:, h, 128 * w : 128 * (w + 1)],
                            start=True,
                            stop=True,
                        )
                    E = e_pool.tile([128, 512], bf16, tag="E")
                    with nc.allow_low_precision(reason="bf16 attn weights"):
                        nc.scalar.activation(
                            E[:], psS[:], mybir.ActivationFunctionType.Exp
                        )
                    E_tiles.append(E)

            def emit_av(v65, w, E_tiles, O_sb):
                """A.V with E stationary: token-major O[q, h, d]; column 64 of
                each head is the softmax denominator (per-partition!), so a
                DVE reciprocal + stride-0-broadcast multiply fuses the
                normalization into the psum eviction."""
                for hb in range(4):
                    E = E_tiles[hb]
                    psO = psO_pool.tile([128, 4, HD + 1], f32, tag="psO")
                    for m in range(4):
                        h = 4 * hb + m
                        nc.tensor.matmul(
                            psO[:, m, :],
                            E[:, 128 * m : 128 * (m + 1)],
                            v65[:, w, h, :],
                            start=True,
                            stop=True,
                        )
                    rq = rq_pool.tile([128, 4], f32, tag="rq")
                    nc.vector.reciprocal(rq[:], psO[:, :, HD])
                    with nc.allow_low_precision(reason="bf16 attn out"):
                        nc.vector.tensor_tensor(
                            O_sb[:, 4 * hb : 4 * hb + 4, :],
                            psO[:, :, 0:HD],
                            rq[:].unsqueeze(-1).broadcast_to([128, 4, HD]),
                            op=mybir.AluOpType.mult,
                        )

            def emit_ot(O_sb, owT):
                """Re-transpose token-major O into owT[c, tok] for the proj."""
                O_flat = O_sb[:].rearrange("p h d -> p (h d)")
                for tb in range(2):
                    ps = psA.tile([128, 512], bf16, tag="psA")
                    for j in range(4):
                        jj = 4 * tb + j
                        nc.tensor.transpose(
                            ps[:, 128 * j : 128 * (j + 1)],
                            O_flat[:, 128 * jj : 128 * (jj + 1)],
                            ident_bf[:],
                        )
                    nc.vector.tensor_copy(
                        owT[:, 4 * tb : 4 * tb + 4, :],
                        ps[:].rearrange("p (c t) -> p c t", t=128),
                    )

            def emit_proj(owT, ih, iw):
                """Out projection (ap=512), evict on the otherwise-idle Pool
                engine (keeps the ACT queue exp-only and DVE out of the psA
                recycling path), DMA-scatter to token order."""
                otile = o_pool.tile([128, C], f32)
                for nk in range(2):
                    ps = psA.tile([128, 512], f32, tag="psA")
                    lo = 512 * nk
                    for k in range(KC):
                        nc.tensor.matmul(
                            ps[:],
                            owT[:, k, :],
                            wp_sb[:, k, lo : lo + 512],
                            start=(k == 0),
                            stop=(k == KC - 1 and not has_projb),
                        )
                    if has_projb:
                        nc.tensor.matmul(
                            ps[:],
                            ones[0:1, 0:128],
                            projb_sb[0:1, lo : lo + 512],
                            start=False,
                            stop=True,
                        )
                    nc.scalar.copy(otile[:, lo : lo + 512], ps[:])
                for tt in range(WT):
                    nc.sync.dma_start(
                        out_v[ih, iw, tt], otile[64 * tt : 64 * (tt + 1), :]
                    )

            # ---- pipelined main loop: attention trails QKV by one group ----
            prev = None  # (qkT, v65, wins) of group g-1
            for g in range(NGRP + 1):
                cur_xT = cur_wins = None
                if g < NGRP:
                    cur_xT, cur_wins = emit_gather(g)
                    qkT = qk_pool.tile([64, 4 * KC, TOKG], bf16)

                pE = [[], []]
                pO = [None, None]
                pOw = [None, None]
                if prev is not None:
                    pqkT, pv65, pwins = prev
                    for w in range(GW):
                        emit_s(pqkT, w, pE[w])
                if g < NGRP:
                    emit_qk(g, cur_xT, qkT, banks=(0, 4, 1, 5))
                if prev is not None:
                    for w in range(GW):
                        pO[w] = o_sb_pool.tile([128, NH, HD], bf16, tag="Osb", name="Osb")
                        emit_av(pv65, w, pE[w], pO[w])
                if g < NGRP:
                    emit_qk(g, cur_xT, qkT, banks=(2, 6))
                if prev is not None:
                    for w in range(GW):
                        pOw[w] = ow_pool.tile([128, KC, 128], bf16, tag="owT", name="owT")
                        emit_ot(pO[w], pOw[w])
                if g < NGRP:
                    emit_qk(g, cur_xT, qkT, banks=(3, 7))
                if prev is not None:
                    for w, (ih, iw) in enumerate(pwins):
                        emit_proj(pOw[w], ih, iw)
                if g < NGRP:
                    v65 = emit_v(g, cur_xT)
                    prev = (qkT, v65, cur_wins)

    _split_drain_waits(nc, mybir)
    return nc


def _get_nc(has_qkvb, has_projb):
    key = (has_qkvb, has_projb)
    if key not in _BUILD_CACHE:
        _BUILD_CACHE[key] = _build(has_qkvb, has_projb)
    return _BUILD_CACHE[key]


def _host_weights(qkv_w, proj_w):
    """Pre-transpose weights, fold the softmax scale into the Q columns,
    cast to bf16, and pre-bank QK ([bank, p, k*256]) / V ([nk, p, k*512])
    so each on-chip weight DMA is a contiguous per-partition copy."""
    wqkvT = qkv_w.T.astype(np.float32)
    wqkvT[:, :C] *= SCALE
    wqkvT = wqkvT.astype(ml_dtypes.bfloat16)
    wt = wqkvT.reshape(KC, 128, 3 * C).transpose(1, 0, 2)
    wqk = np.ascontiguousarray(
        wt[:, :, : 2 * C].reshape(128, KC, 8, 256).transpose(2, 0, 1, 3)
    ).reshape(8, 128, KC * 256)
    wv = np.ascontiguousarray(
        wt[:, :, 2 * C :].reshape(128, KC, 2, 512).transpose(2, 0, 1, 3)
    ).reshape(2, 128, KC * 512)
    return wqk, wv, np.ascontiguousarray(proj_w.T).astype(ml_dtypes.bfloat16)


def _host_gather_transpose(x):
    """x [B, N, C] -> per (b, it) slab: window-gathered, channel-transposed
    bf16 [NWIN, 128, KC*128]: out[win, 128*?+p? ...] — precisely
    out[win, p, 128*k + t] = x[b, slab + token(win, t), 128*k + p], with
    token (tt, hh, ww) order inside the window and win = 4*ih + iw."""
    xb = x.astype(ml_dtypes.bfloat16)
    xb = xb.reshape(B, T // WT, WT, 4, WH, 4, WW, KC, 128)
    # -> [b, it, ih, iw, (tt hh ww)=t, k, p] -> [b, it, win, p, k, t]
    xb = xb.transpose(0, 1, 3, 5, 2, 4, 6, 7, 8)
    xb = xb.reshape(B, T // WT, NWIN, M, KC, 128)
    xb = np.ascontiguousarray(xb.transpose(0, 1, 2, 5, 4, 3))
    return xb.reshape(B, T // WT, NWIN, 128, KC * 128)


def make_in_maps(x, qkv_w, qkv_b, proj_w, proj_b):
    has_qkvb = bool(np.any(qkv_b))
    has_projb = bool(np.any(proj_b))
    wqk, wv, projT = _host_weights(qkv_w, proj_w)
    xswT = _host_gather_transpose(x)
    ident = np.eye(128, dtype=np.float32).astype(ml_dtypes.bfloat16)
    in_maps = []
    for core in range(NCORES):
        b, it = divmod(core, T // WT)
        im = {
            "xswT": xswT[b, it],
            "wqk": wqk,
            "wv": wv,
            "projT": projT,
            "ident": ident,
        }
        if has_qkvb:
            im["qkvb"] = qkv_b.reshape(1, 3 * C).astype(ml_dtypes.bfloat16)
        if has_projb:
            im["projb"] = proj_b.reshape(1, C).astype(ml_dtypes.bfloat16)
        in_maps.append(im)
    return in_maps, has_qkvb, has_projb


def kernel(x, qkv_w, qkv_b, proj_w, proj_b, t, h, w, **_unused):
    from concourse.bass_utils import run_bass_kernel_spmd

    x = np.asarray(x, dtype=np.float32)
    qkv_w = np.asarray(qkv_w, dtype=np.float32)
    qkv_b = np.asarray(qkv_b, dtype=np.float32)
    proj_w = np.asarray(proj_w, dtype=np.float32)
    proj_b = np.asarray(proj_b, dtype=np.float32)
    assert x.shape == (B, N, C), x.shape
    assert int(t) == T and int(h) == H and int(w) == W

    in_maps, has_qkvb, has_projb = make_in_maps(x, qkv_w, qkv_b, proj_w, proj_b)
    nc = _get_nc(has_qkvb, has_projb)

    res = run_bass_kernel_spmd(nc, in_maps, core_ids=list(range(NCORES)))

    y = np.empty((B, N, C), dtype=np.float32)
    for core in range(NCORES):
        b, it = divmod(core, T // WT)
        y[b, it * SLAB : (it + 1) * SLAB, :] = res.results[core]["out"]
    return y
